# revision 12
# baseline (speedup 1.0000x reference)
"""Trainium2 Bass kernel for nn_NodeFeatByVN (gnn_message_passing).

Math insight: in the reference,
    info_vec[n,i] = node_pos[n] - vn_pos[batch[n], 3i:3i+3]
so pairwise differences info_vec[n,i]-info_vec[n,j] depend only on vn_pos rows
of group batch[n]; node_pos cancels.  Hence

    out[n] = MLP(normalized_cdist(vn_pos[batch[n]]))  =  table[batch[n]]

with an 8192x64 table.  Each core computes its slice of the table on-chip
(~1100 groups), replicates each row 32x in SBUF (8KB per group), and expands
it into the 1M-row output with indirect-DMA scatters (SBUF -> DRAM):

  - batch is sorted, so the output is a sequence of per-group runs; shards
    are cut at run boundaries so no run is split across cores;
  - each run of c rows is covered by ceil(c/32) 32-row (8KB) block writes,
    the last block shifted back to end exactly at the run end (overlapping
    rows are rewritten with identical bytes - benign);
  - destination row offsets are host-precomputed input DATA; unused slots
    hold an out-of-bounds sentinel, skipped via bounds_check/oob_is_err=False;
  - Tile's write-after-write serialization between scatters (false dep:
    disjoint rows) is downgraded to a no-sync ordering edge, which lets the
    DMA engines pipeline at full HBM write bandwidth.

All per-core variability (table slice, offsets, shard size) is input data,
so one SPMD program runs on all 8 cores.  Pathological inputs (any run
shorter than 32 rows, wider group span, more blocks per run) are handled by
recompiling with small-block classes enabled / bigger NT / bigger R - same
program on all cores, chosen from global maxima.

Sharding: data-parallel over nodes (~125k rows/core); vn_pos slice + MLP
weights replicated per core; no collectives.
"""

import sys
import types

import numpy as np
from contextlib import ExitStack

import bass_rust
import concourse.bass as bass
import concourse.bacc as bacc
import concourse.tile as tile
import concourse.mybir as mybir
from concourse.bass_utils import run_bass_kernel_spmd
from concourse.masks import make_identity


def _install_ntff_shim():
    """Best-effort: make trace=True/BASS_TRACE work in containers whose axon
    boot lacks the `antenv.axon_hooks` registry module."""
    try:
        import antenv.axon_hooks  # noqa: F401
        return
    except ImportError:
        pass
    try:
        mod = types.ModuleType("antenv.axon_hooks")
        mod._hook = None
        mod.set_axon_ntff_profile_hook = lambda h: setattr(mod, "_hook", h)
        mod.get_axon_ntff_profile_hook = lambda: mod._hook
        sys.modules["antenv.axon_hooks"] = mod
        try:
            import antenv
            antenv.axon_hooks = mod
        except ImportError:
            pass
        from trn_agent_boot.trn_boot import _ntff_profile_via_ctypes
        mod.set_axon_ntff_profile_hook(
            _ntff_profile_via_ctypes("/opt/axon/libaxon_pjrt.so"))
        import concourse.bass_utils as _bu
        _real_upload = _bu.upload_artifacts

        def _safe_upload(tmpdir):
            try:
                return _real_upload(tmpdir)
            except Exception:
                return f"local:{tmpdir}"

        _bu.upload_artifacts = _safe_upload
    except Exception:
        pass


_install_ntff_shim()

F32 = mybir.dt.float32
I32 = mybir.dt.int32
AF = mybir.ActivationFunctionType
OP = mybir.AluOpType

NCORES = 8
G = 8192
HID = 64
BLK = 32               # rows per full scatter block (8KB)
REP = 32               # table-row replicas kept in SBUF (>= BLK)
SMALL_CLASSES = (16, 8, 4, 2, 1)


def build_program(nsh_max, nt, r32, small_mode):
    """One SPMD program.

    nsh_max: padded output rows per core (actual shard size <= nsh_max).
    nt:      group tiles of 128 (table slice size nt*128 groups).
    r32:     full-block slots per group.
    small_mode: also emit 2 scatter slots per small class per tile
                (only needed when some run has fewer than BLK rows).
    Inputs per core: vn_slice [nt*128,12], w1t17 [17,64] (W1.T + b1 row),
    w2t [64,64] (W2.T), b2rep [128,64], blkidx [nt,128,r32],
    smallidx [nt,128,2*len(SMALL_CLASSES)] (only in small_mode).
    Output: out [nsh_max, 64].
    """
    nc = bacc.Bacc("TRN2", target_bir_lowering=False, debug=False)
    vn = nc.dram_tensor("vn_slice", [nt * 128, 12], F32, kind="ExternalInput").ap()
    w1t = nc.dram_tensor("w1t17", [17, HID], F32, kind="ExternalInput").ap()
    w2t = nc.dram_tensor("w2t", [HID, HID], F32, kind="ExternalInput").ap()
    b2r = nc.dram_tensor("b2rep", [128, HID], F32, kind="ExternalInput").ap()
    bidx = nc.dram_tensor("blkidx", [nt, 128, r32 + 1], I32, kind="ExternalInput").ap()
    if small_mode:
        nsm = 2 * len(SMALL_CLASSES)
        sidx = nc.dram_tensor("smallidx", [nt, 128, nsm], I32, kind="ExternalInput").ap()
    out = nc.dram_tensor("out", [nsh_max, HID], F32, kind="ExternalOutput").ap()

    scatters = []
    with ExitStack() as ctx:
        tc = ctx.enter_context(tile.TileContext(nc))
        const = ctx.enter_context(tc.tile_pool(name="const", bufs=1))
        work = ctx.enter_context(tc.tile_pool(name="work", bufs=4))
        repp = ctx.enter_context(tc.tile_pool(name="rep", bufs=4))
        psum = ctx.enter_context(tc.tile_pool(name="psum", bufs=2, space="PSUM"))

        ident = const.tile([128, 128], F32)
        make_identity(nc, ident[:])
        w1s = const.tile([17, HID], F32)
        nc.sync.dma_start(w1s[:], w1t)
        w2s = const.tile([HID, HID], F32)
        nc.sync.dma_start(w2s[:], w2t)
        b2s = const.tile([128, HID], F32)
        nc.sync.dma_start(b2s[:], b2r)

        for i in range(nt):
            vt = work.tile([128, 12], F32)
            nc.sync.dma_start(vt[:], vn[i * 128:(i + 1) * 128, :])
            v4 = vt[:].rearrange("p (i k) -> p i k", k=3)
            a_ap = v4[:, :, None, :].to_broadcast([128, 4, 4, 3])
            b_ap = v4[:, None, :, :].to_broadcast([128, 4, 4, 3])

            # diff[p,i,j,k] = v[p,i,k]-v[p,j,k];  sq = diff^2
            diff = work.tile([128, 48], F32)
            d3 = diff[:].rearrange("p (i j k) -> p i j k", j=4, k=3)
            nc.vector.tensor_tensor(out=d3, in0=a_ap, in1=b_ap, op=OP.subtract)
            sq = work.tile([128, 48], F32)
            nc.vector.tensor_tensor(out=sq[:], in0=diff[:], in1=diff[:], op=OP.mult)

            # d2[p,ij] = sum_k sq;  s2 = sum d2 = ||dist||^2;  dist = sqrt(d2)
            sqv = sq[:].rearrange("p (ij k) -> p ij k", k=3)
            d2 = work.tile([128, 16], F32)
            nc.vector.tensor_tensor(out=d2[:], in0=sqv[:, :, 0], in1=sqv[:, :, 1], op=OP.add)
            nc.vector.tensor_tensor(out=d2[:], in0=d2[:], in1=sqv[:, :, 2], op=OP.add)
            s2 = work.tile([128, 1], F32)
            nc.vector.reduce_sum(out=s2[:], in_=d2[:], axis=mybir.AxisListType.X)
            d17 = work.tile([128, 17], F32)
            nc.scalar.activation(out=d17[:, 0:16], in_=d2[:], func=AF.Sqrt)
            nc.scalar.activation(out=d17[:, 16:17], in_=s2[:], func=AF.Sqrt)
            # col 16 <- norm+0.001: the b1-row coefficient in mm1; rn = 1/(norm+eps)
            nc.vector.tensor_scalar_add(out=d17[:, 16:17], in0=d17[:, 16:17], scalar1=0.001)
            rn = work.tile([128, 1], F32)
            nc.vector.reciprocal(out=rn[:], in_=d17[:, 16:17])

            # Pre-scale by rnorm (col 16 becomes rn*(norm+eps) ~= 1.0, the b1
            # coefficient), so mm1 = nd @ W1.T + b1 with no epilogue scale.
            d17s = work.tile([128, 17], F32)
            nc.vector.tensor_scalar_mul(out=d17s[:], in0=d17[:], scalar1=rn[:])

            # h1T [64,128] = W1 @ nd.T + b1 (transposed orientation avoids a
            # second PE transpose);  hsT = h1T * sigmoid(h1T)  (silu)
            tp1 = psum.tile([17, 128], F32)
            nc.tensor.transpose(out=tp1[:], in_=d17s[:], identity=ident[:])
            l1 = work.tile([17, 128], F32)
            nc.vector.tensor_copy(out=l1[:], in_=tp1[:])
            h1t = psum.tile([HID, 128], F32)
            nc.tensor.matmul(out=h1t[:], lhsT=w1s[:], rhs=l1[:], start=True, stop=True)
            hgt = work.tile([HID, 128], F32)
            nc.scalar.activation(out=hgt[:], in_=h1t[:], func=AF.Sigmoid)
            hst = work.tile([HID, 128], F32)
            nc.vector.tensor_tensor(out=hst[:], in0=h1t[:], in1=hgt[:], op=OP.mult)

            # o = h @ W2.T  (lhsT = hsT directly);  bias-add while replicating
            o2 = psum.tile([128, HID], F32)
            nc.tensor.matmul(out=o2[:], lhsT=hst[:], rhs=w2s[:], start=True, stop=True)

            rep = repp.tile([128, REP * HID], F32)
            repv = rep[:].rearrange("p (r d) -> p r d", r=REP)
            nc.vector.tensor_tensor(
                out=repv,
                in0=o2[:][:, None, :].to_broadcast([128, REP, HID]),
                in1=b2s[:][:, None, :].to_broadcast([128, REP, HID]),
                op=OP.add,
            )

            # 32-row block scatters, one [128,1]-offset indirect per slot;
            # slot r32 (last) is the 16-row tail class.
            ixb = work.tile([128, r32 + 1], I32)
            nc.sync.dma_start(ixb[:], bidx[i])
            for rr in range(r32 + 1):
                blk_rows = BLK if rr < r32 else 16
                h = nc.gpsimd.indirect_dma_start(
                    out=out,
                    out_offset=bass.IndirectOffsetOnAxis(ap=ixb[:, rr:rr + 1], axis=0),
                    in_=rep[:, :blk_rows * HID],
                    in_offset=None,
                    bounds_check=nsh_max - 1,
                    oob_is_err=False,
                )
                scatters.append(h.ins)
            if small_mode:
                ixs = work.tile([128, nsm], I32)
                nc.sync.dma_start(ixs[:], sidx[i])
                for ci, csz in enumerate(SMALL_CLASSES):
                    for rr in range(2):
                        col = 2 * ci + rr
                        h = nc.gpsimd.indirect_dma_start(
                            out=out,
                            out_offset=bass.IndirectOffsetOnAxis(
                                ap=ixs[:, col:col + 1], axis=0),
                            in_=rep[:, :csz * HID],
                            in_offset=None,
                            bounds_check=nsh_max - 1,
                            oob_is_err=False,
                        )
                        scatters.append(h.ins)

        # Downgrade scatter->scatter sync deps (false WAW: rows are disjoint
        # by construction, or carry identical bytes) to ordering-only edges so
        # the DMA queue pipelines at full write bandwidth.
        names = {s.name for s in scatters}
        for s in scatters:
            syncs = list(s.sync_dependency_names())
            demote = [d for d in syncs if d in names]
            if demote:
                s.set_sync_dependencies(bass_rust.InstructionNameOrderedSet(
                    [d for d in syncs if d not in names]))
                s.set_nosync_dependencies(bass_rust.InstructionNameOrderedSet(
                    list(s.nosync_dependency_names()) + demote))
    nc.compile()
    return nc


def _shard_cuts(b, ncores):
    """Cut points aligned to run starts, near c*N/ncores."""
    n = b.shape[0]
    cuts = [0]
    for c in range(1, ncores):
        t = c * n // ncores
        g = b[t]
        # first index of the run containing t, or of the next run
        lo = int(np.searchsorted(b, g, side="left"))
        hi = int(np.searchsorted(b, g, side="right"))
        cuts.append(lo if t - lo <= hi - t else hi)
    cuts.append(n)
    return cuts


def build_host_inputs(vn_pos, batch, W1, b1, W2, b2, ncores, g=G):
    b = np.ascontiguousarray(batch).astype(np.int64)
    n = b.shape[0]
    counts = np.bincount(b, minlength=g)
    ends_g = np.cumsum(counts)
    starts_g = ends_g - counts

    cuts = _shard_cuts(b, ncores)
    sizes = [cuts[c + 1] - cuts[c] for c in range(ncores)]
    nsh_max = -(-max(sizes) // 512) * 512

    g0s, spans = [], []
    for c in range(ncores):
        lo, hi = cuts[c], cuts[c + 1]
        g0, g1 = int(b[lo]), int(b[hi - 1])
        g0s.append(g0)
        spans.append(g1 - g0 + 1)
    nt = max(1, -(-max(spans) // 128))
    ng = nt * 128

    cmax = int(counts.max())
    small_mode = bool((counts[(counts > 0)] < BLK).any())
    r32 = max(1, -(-cmax // BLK))

    w1t17 = np.concatenate([W1.T, b1[None, :]], axis=0).astype(np.float32)
    w2t = np.ascontiguousarray(W2.T).astype(np.float32)
    b2rep = np.tile(b2[None, :], (128, 1)).astype(np.float32)

    in_maps = []
    for c in range(ncores):
        lo, hi = cuts[c], cuts[c + 1]
        size_c = hi - lo
        g0 = g0s[c]
        gidx = np.arange(g0, g0 + ng)
        valid = gidx < g
        # full runs (never split across cores): local [s, e) per group
        s = np.zeros(ng, np.int64)
        e = np.zeros(ng, np.int64)
        s[valid] = np.clip(starts_g[gidx[valid]], lo, hi) - lo
        e[valid] = np.clip(ends_g[gidx[valid]], lo, hi) - lo
        c_run = e - s

        vn_slice = np.zeros((ng, 12), np.float32)
        hi_g = min(g0 + ng, g)
        vn_slice[: hi_g - g0] = vn_pos[g0:hi_g]

        # full 32-row blocks: off[r] = min(s+32r, e-32); when the tail
        # remainder is in (0,16] the last full block is not shifted and a
        # 16-row tail block at e-16 (extra column) covers the remainder.
        rem = np.where(c_run >= BLK, c_run % BLK, 0)
        small_tail = (c_run >= BLK) & (rem > 0) & (rem <= 16)
        nb = np.where(c_run >= BLK,
                      np.where(small_tail, c_run // BLK, -(-c_run // BLK)), 0)
        ar = np.arange(r32)[None, :]
        off = np.minimum(s[:, None] + BLK * ar, (e - BLK)[:, None])
        blk32 = np.where(ar < nb[:, None], off, nsh_max)
        tail16 = np.where(small_tail, e - 16, nsh_max)
        blkidx = np.concatenate([blk32, tail16[:, None]], axis=1).astype(np.int32)

        im = {
            "vn_slice": vn_slice,
            "w1t17": w1t17,
            "w2t": w2t,
            "b2rep": b2rep,
            "blkidx": blkidx.reshape(nt, 128, r32 + 1),
            "_size": size_c,
        }
        if small_mode:
            nsm = 2 * len(SMALL_CLASSES)
            smallidx = np.full((ng, nsm), nsh_max, np.int64)
            sml = np.nonzero((c_run > 0) & (c_run < BLK))[0]
            for l in sml:
                cc = int(c_run[l])
                k = 1 << (cc.bit_length() - 1)  # largest pow2 <= cc
                ci = SMALL_CLASSES.index(k)
                smallidx[l, 2 * ci] = s[l]
                smallidx[l, 2 * ci + 1] = e[l] - k
            im["smallidx"] = smallidx.reshape(nt, 128, nsm).astype(np.int32)
        in_maps.append(im)
    return in_maps, dict(nsh_max=nsh_max, nt=nt, r32=r32, small_mode=small_mode)


_prog_cache = {}


def kernel(node_feat, node_pos, vn_pos, batch, W1, b1, W2, b2, **_unused):
    """Full inputs in, full output out.  node_feat/node_pos are mathematically
    unused (node position cancels in the pairwise distances)."""
    del node_feat, node_pos
    vn_pos = np.asarray(vn_pos, np.float32)
    batch = np.asarray(batch)
    W1 = np.asarray(W1, np.float32)
    b1 = np.asarray(b1, np.float32)
    W2 = np.asarray(W2, np.float32)
    b2 = np.asarray(b2, np.float32)

    in_maps, shape = build_host_inputs(vn_pos, batch, W1, b1, W2, b2, NCORES)
    sizes = [im.pop("_size") for im in in_maps]
    key = tuple(sorted(shape.items()))
    if key not in _prog_cache:
        _prog_cache[key] = build_program(**shape)
    nc = _prog_cache[key]

    res = run_bass_kernel_spmd(nc, in_maps, core_ids=list(range(NCORES)))
    global LAST_RESULT
    LAST_RESULT = res
    return np.concatenate(
        [res.results[c]["out"][:sizes[c]] for c in range(NCORES)], axis=0)


LAST_RESULT = None


# revision 16
# speedup vs baseline: 1.0430x; 1.0430x over previous
"""Trainium2 Bass kernel for nn_NodeFeatByVN (gnn_message_passing).

Math insight: in the reference,
    info_vec[n,i] = node_pos[n] - vn_pos[batch[n], 3i:3i+3]
so pairwise differences info_vec[n,i]-info_vec[n,j] depend only on vn_pos rows
of group batch[n]; node_pos cancels.  Hence

    out[n] = MLP(normalized_cdist(vn_pos[batch[n]]))  =  table[batch[n]]

with an 8192x64 table.  Each core computes its slice of the table on-chip
(~1100 groups), replicates each row 32x in SBUF (8KB per group), and expands
it into the 1M-row output with indirect-DMA scatters (SBUF -> DRAM):

  - batch is sorted, so the output is a sequence of per-group runs; shards
    are cut at run boundaries so no run is split across cores;
  - each run of c rows is covered by ceil(c/32) 32-row (8KB) block writes,
    the last block shifted back to end exactly at the run end (overlapping
    rows are rewritten with identical bytes - benign);
  - destination row offsets are host-precomputed input DATA; unused slots
    hold an out-of-bounds sentinel, skipped via bounds_check/oob_is_err=False;
  - Tile's write-after-write serialization between scatters (false dep:
    disjoint rows) is downgraded to a no-sync ordering edge, which lets the
    DMA engines pipeline at full HBM write bandwidth.

All per-core variability (table slice, offsets, shard size) is input data,
so one SPMD program runs on all 8 cores.  Pathological inputs (any run
shorter than 32 rows, wider group span, more blocks per run) are handled by
recompiling with small-block classes enabled / bigger NT / bigger R - same
program on all cores, chosen from global maxima.

Sharding: data-parallel over nodes (~125k rows/core); vn_pos slice + MLP
weights replicated per core; no collectives.
"""

import sys
import types

import numpy as np
from contextlib import ExitStack

import bass_rust
import concourse.bass as bass
import concourse.bacc as bacc
import concourse.tile as tile
import concourse.mybir as mybir
from concourse.bass_utils import run_bass_kernel_spmd
from concourse.masks import make_identity


def _install_ntff_shim():
    """Best-effort: make trace=True/BASS_TRACE work in containers whose axon
    boot lacks the `antenv.axon_hooks` registry module."""
    try:
        import antenv.axon_hooks  # noqa: F401
        return
    except ImportError:
        pass
    try:
        mod = types.ModuleType("antenv.axon_hooks")
        mod._hook = None
        mod.set_axon_ntff_profile_hook = lambda h: setattr(mod, "_hook", h)
        mod.get_axon_ntff_profile_hook = lambda: mod._hook
        sys.modules["antenv.axon_hooks"] = mod
        try:
            import antenv
            antenv.axon_hooks = mod
        except ImportError:
            pass
        from trn_agent_boot.trn_boot import _ntff_profile_via_ctypes
        mod.set_axon_ntff_profile_hook(
            _ntff_profile_via_ctypes("/opt/axon/libaxon_pjrt.so"))
        import concourse.bass_utils as _bu
        _real_upload = _bu.upload_artifacts

        def _safe_upload(tmpdir):
            try:
                return _real_upload(tmpdir)
            except Exception:
                return f"local:{tmpdir}"

        _bu.upload_artifacts = _safe_upload
    except Exception:
        pass


_install_ntff_shim()

F32 = mybir.dt.float32
I32 = mybir.dt.int32
AF = mybir.ActivationFunctionType
OP = mybir.AluOpType

NCORES = 8
G = 8192
HID = 64
BLK = 32               # rows per full scatter block (8KB)
REP = 32               # table-row replicas kept in SBUF (>= BLK)
SMALL_CLASSES = (16, 8, 4, 2, 1)


def build_program(nsh_max, nt, r32, small_mode):
    """One SPMD program.

    nsh_max: padded output rows per core (actual shard size <= nsh_max).
    nt:      group tiles of 128 (table slice size nt*128 groups).
    r32:     full-block slots per group.
    small_mode: also emit 2 scatter slots per small class per tile
                (only needed when some run has fewer than BLK rows).
    Inputs per core: vn_slice [nt*128,12], w1t17 [17,64] (W1.T + b1 row),
    w2t [64,64] (W2.T), b2rep [128,64], blkidx [nt,128,r32],
    smallidx [nt,128,2*len(SMALL_CLASSES)] (only in small_mode).
    Output: out [nsh_max, 64].
    """
    nc = bacc.Bacc("TRN2", target_bir_lowering=False, debug=False)
    vn = nc.dram_tensor("vn_slice", [nt * 128, 12], F32, kind="ExternalInput").ap()
    w1t = nc.dram_tensor("w1t17", [17, HID], F32, kind="ExternalInput").ap()
    w2t = nc.dram_tensor("w2t", [HID, HID], F32, kind="ExternalInput").ap()
    b2r = nc.dram_tensor("b2rep", [128, HID], F32, kind="ExternalInput").ap()
    bidx = nc.dram_tensor("blkidx", [nt, 128, r32 + 1], I32, kind="ExternalInput").ap()
    if small_mode:
        nsm = 2 * len(SMALL_CLASSES)
        sidx = nc.dram_tensor("smallidx", [nt, 128, nsm], I32, kind="ExternalInput").ap()
    out = nc.dram_tensor("out", [nsh_max, HID], F32, kind="ExternalOutput").ap()

    scatters = []
    with ExitStack() as ctx:
        tc = ctx.enter_context(tile.TileContext(nc))
        const = ctx.enter_context(tc.tile_pool(name="const", bufs=1))
        work = ctx.enter_context(tc.tile_pool(name="work", bufs=4))
        repp = ctx.enter_context(tc.tile_pool(name="rep", bufs=4))
        psum = ctx.enter_context(tc.tile_pool(name="psum", bufs=2, space="PSUM"))

        ident = const.tile([128, 128], F32)
        make_identity(nc, ident[:])
        w1s = const.tile([17, HID], F32)
        nc.sync.dma_start(w1s[:], w1t)
        w2s = const.tile([HID, HID], F32)
        nc.sync.dma_start(w2s[:], w2t)
        b2s = const.tile([128, HID], F32)
        nc.sync.dma_start(b2s[:], b2r)

        for i in range(nt):
            vt = work.tile([128, 12], F32)
            nc.sync.dma_start(vt[:], vn[i * 128:(i + 1) * 128, :])
            v4 = vt[:].rearrange("p (i k) -> p i k", k=3)
            a_ap = v4[:, :, None, :].to_broadcast([128, 4, 4, 3])
            b_ap = v4[:, None, :, :].to_broadcast([128, 4, 4, 3])

            # diff[p,i,j,k] = v[p,i,k]-v[p,j,k];  sq = diff^2
            diff = work.tile([128, 48], F32)
            d3 = diff[:].rearrange("p (i j k) -> p i j k", j=4, k=3)
            nc.vector.tensor_tensor(out=d3, in0=a_ap, in1=b_ap, op=OP.subtract)
            sq = work.tile([128, 48], F32)
            nc.vector.tensor_tensor(out=sq[:], in0=diff[:], in1=diff[:], op=OP.mult)

            # d2[p,ij] = sum_k sq;  s2 = sum d2 = ||dist||^2;  dist = sqrt(d2)
            sqv = sq[:].rearrange("p (ij k) -> p ij k", k=3)
            d2 = work.tile([128, 16], F32)
            nc.vector.reduce_sum(out=d2[:][:, :, None], in_=sqv, axis=mybir.AxisListType.X)
            s2 = work.tile([128, 1], F32)
            nc.vector.reduce_sum(out=s2[:], in_=sq[:], axis=mybir.AxisListType.X)
            d17 = work.tile([128, 17], F32)
            nc.scalar.activation(out=d17[:, 0:16], in_=d2[:], func=AF.Sqrt)
            nc.scalar.activation(out=d17[:, 16:17], in_=s2[:], func=AF.Sqrt)
            # col 16 <- norm+0.001: the b1-row coefficient in mm1; rn = 1/(norm+eps)
            nc.vector.tensor_scalar_add(out=d17[:, 16:17], in0=d17[:, 16:17], scalar1=0.001)
            rn = work.tile([128, 1], F32)
            nc.vector.reciprocal(out=rn[:], in_=d17[:, 16:17])

            # h = silu(rn*(dist@W1.T + (norm+eps)*b1)) = silu(nd@W1.T + b1)
            tp1 = psum.tile([17, 128], F32)
            nc.tensor.transpose(out=tp1[:], in_=d17[:], identity=ident[:])
            l1 = work.tile([17, 128], F32)
            nc.vector.tensor_copy(out=l1[:], in_=tp1[:])
            h1 = psum.tile([128, HID], F32)
            nc.tensor.matmul(out=h1[:], lhsT=l1[:], rhs=w1s[:], start=True, stop=True)
            hx = work.tile([128, HID], F32)
            nc.vector.tensor_scalar_mul(out=hx[:], in0=h1[:], scalar1=rn[:])
            hg = work.tile([128, HID], F32)
            nc.scalar.activation(out=hg[:], in_=h1[:], func=AF.Sigmoid, scale=rn[:])
            hs = work.tile([128, HID], F32)
            nc.vector.tensor_tensor(out=hs[:], in0=hx[:], in1=hg[:], op=OP.mult)

            # o = h @ W2.T;  bias-add while replicating REP times per partition
            tp2 = psum.tile([HID, 128], F32)
            nc.tensor.transpose(out=tp2[:], in_=hs[:], identity=ident[:])
            l2 = work.tile([HID, 128], F32)
            nc.vector.tensor_copy(out=l2[:], in_=tp2[:])
            o2 = psum.tile([128, HID], F32)
            nc.tensor.matmul(out=o2[:], lhsT=l2[:], rhs=w2s[:], start=True, stop=True)

            rep = repp.tile([128, REP * HID], F32)
            repv = rep[:].rearrange("p (r d) -> p r d", r=REP)
            nc.vector.tensor_tensor(
                out=repv,
                in0=o2[:][:, None, :].to_broadcast([128, REP, HID]),
                in1=b2s[:][:, None, :].to_broadcast([128, REP, HID]),
                op=OP.add,
            )

            # 32-row block scatters, one [128,1]-offset indirect per slot;
            # slot r32 (last) is the 16-row tail class.
            ixb = work.tile([128, r32 + 1], I32)
            nc.sync.dma_start(ixb[:], bidx[i])
            for rr in range(r32 + 1):
                blk_rows = BLK if rr < r32 else 16
                h = nc.gpsimd.indirect_dma_start(
                    out=out,
                    out_offset=bass.IndirectOffsetOnAxis(ap=ixb[:, rr:rr + 1], axis=0),
                    in_=rep[:, :blk_rows * HID],
                    in_offset=None,
                    bounds_check=nsh_max - 1,
                    oob_is_err=False,
                )
                scatters.append(h.ins)
            if small_mode:
                ixs = work.tile([128, nsm], I32)
                nc.sync.dma_start(ixs[:], sidx[i])
                for ci, csz in enumerate(SMALL_CLASSES):
                    for rr in range(2):
                        col = 2 * ci + rr
                        h = nc.gpsimd.indirect_dma_start(
                            out=out,
                            out_offset=bass.IndirectOffsetOnAxis(
                                ap=ixs[:, col:col + 1], axis=0),
                            in_=rep[:, :csz * HID],
                            in_offset=None,
                            bounds_check=nsh_max - 1,
                            oob_is_err=False,
                        )
                        scatters.append(h.ins)

        # Downgrade scatter->scatter sync deps (false WAW: rows are disjoint
        # by construction, or carry identical bytes) to ordering-only edges so
        # the DMA queue pipelines at full write bandwidth.
        names = {s.name for s in scatters}
        for s in scatters:
            syncs = list(s.sync_dependency_names())
            demote = [d for d in syncs if d in names]
            if demote:
                s.set_sync_dependencies(bass_rust.InstructionNameOrderedSet(
                    [d for d in syncs if d not in names]))
                s.set_nosync_dependencies(bass_rust.InstructionNameOrderedSet(
                    list(s.nosync_dependency_names()) + demote))
    nc.compile()
    return nc


def _shard_cuts(b, ncores):
    """Cut points aligned to run starts, near c*N/ncores."""
    n = b.shape[0]
    cuts = [0]
    for c in range(1, ncores):
        t = c * n // ncores
        g = b[t]
        # first index of the run containing t, or of the next run
        lo = int(np.searchsorted(b, g, side="left"))
        hi = int(np.searchsorted(b, g, side="right"))
        cuts.append(lo if t - lo <= hi - t else hi)
    cuts.append(n)
    return cuts


def build_host_inputs(vn_pos, batch, W1, b1, W2, b2, ncores, g=G):
    b = np.ascontiguousarray(batch).astype(np.int64)
    n = b.shape[0]
    counts = np.bincount(b, minlength=g)
    ends_g = np.cumsum(counts)
    starts_g = ends_g - counts

    cuts = _shard_cuts(b, ncores)
    sizes = [cuts[c + 1] - cuts[c] for c in range(ncores)]
    nsh_max = -(-max(sizes) // 512) * 512

    g0s, spans = [], []
    for c in range(ncores):
        lo, hi = cuts[c], cuts[c + 1]
        g0, g1 = int(b[lo]), int(b[hi - 1])
        g0s.append(g0)
        spans.append(g1 - g0 + 1)
    nt = max(1, -(-max(spans) // 128))
    ng = nt * 128

    cmax = int(counts.max())
    small_mode = bool((counts[(counts > 0)] < BLK).any())
    r32 = max(1, -(-cmax // BLK))

    w1t17 = np.concatenate([W1.T, b1[None, :]], axis=0).astype(np.float32)
    w2t = np.ascontiguousarray(W2.T).astype(np.float32)
    b2rep = np.tile(b2[None, :], (128, 1)).astype(np.float32)

    in_maps = []
    for c in range(ncores):
        lo, hi = cuts[c], cuts[c + 1]
        size_c = hi - lo
        g0 = g0s[c]
        gidx = np.arange(g0, g0 + ng)
        valid = gidx < g
        # full runs (never split across cores): local [s, e) per group
        s = np.zeros(ng, np.int64)
        e = np.zeros(ng, np.int64)
        s[valid] = np.clip(starts_g[gidx[valid]], lo, hi) - lo
        e[valid] = np.clip(ends_g[gidx[valid]], lo, hi) - lo
        c_run = e - s

        vn_slice = np.zeros((ng, 12), np.float32)
        hi_g = min(g0 + ng, g)
        vn_slice[: hi_g - g0] = vn_pos[g0:hi_g]

        # full 32-row blocks: off[r] = min(s+32r, e-32); when the tail
        # remainder is in (0,16] the last full block is not shifted and a
        # 16-row tail block at e-16 (extra column) covers the remainder.
        rem = np.where(c_run >= BLK, c_run % BLK, 0)
        small_tail = (c_run >= BLK) & (rem > 0) & (rem <= 16)
        nb = np.where(c_run >= BLK,
                      np.where(small_tail, c_run // BLK, -(-c_run // BLK)), 0)
        ar = np.arange(r32)[None, :]
        off = np.minimum(s[:, None] + BLK * ar, (e - BLK)[:, None])
        blk32 = np.where(ar < nb[:, None], off, nsh_max)
        tail16 = np.where(small_tail, e - 16, nsh_max)
        blkidx = np.concatenate([blk32, tail16[:, None]], axis=1).astype(np.int32)

        im = {
            "vn_slice": vn_slice,
            "w1t17": w1t17,
            "w2t": w2t,
            "b2rep": b2rep,
            "blkidx": blkidx.reshape(nt, 128, r32 + 1),
            "_size": size_c,
        }
        if small_mode:
            nsm = 2 * len(SMALL_CLASSES)
            smallidx = np.full((ng, nsm), nsh_max, np.int64)
            sml = np.nonzero((c_run > 0) & (c_run < BLK))[0]
            for l in sml:
                cc = int(c_run[l])
                k = 1 << (cc.bit_length() - 1)  # largest pow2 <= cc
                ci = SMALL_CLASSES.index(k)
                smallidx[l, 2 * ci] = s[l]
                smallidx[l, 2 * ci + 1] = e[l] - k
            im["smallidx"] = smallidx.reshape(nt, 128, nsm).astype(np.int32)
        in_maps.append(im)
    return in_maps, dict(nsh_max=nsh_max, nt=nt, r32=r32, small_mode=small_mode)


_prog_cache = {}


def kernel(node_feat, node_pos, vn_pos, batch, W1, b1, W2, b2, **_unused):
    """Full inputs in, full output out.  node_feat/node_pos are mathematically
    unused (node position cancels in the pairwise distances)."""
    del node_feat, node_pos
    vn_pos = np.asarray(vn_pos, np.float32)
    batch = np.asarray(batch)
    W1 = np.asarray(W1, np.float32)
    b1 = np.asarray(b1, np.float32)
    W2 = np.asarray(W2, np.float32)
    b2 = np.asarray(b2, np.float32)

    in_maps, shape = build_host_inputs(vn_pos, batch, W1, b1, W2, b2, NCORES)
    sizes = [im.pop("_size") for im in in_maps]
    key = tuple(sorted(shape.items()))
    if key not in _prog_cache:
        _prog_cache[key] = build_program(**shape)
    nc = _prog_cache[key]

    res = run_bass_kernel_spmd(nc, in_maps, core_ids=list(range(NCORES)))
    global LAST_RESULT
    LAST_RESULT = res
    return np.concatenate(
        [res.results[c]["out"][:sizes[c]] for c in range(NCORES)], axis=0)


LAST_RESULT = None
